# revision 9
# baseline (speedup 1.0000x reference)
"""EdgeConv (gnn_message_passing) Trainium2 Bass kernel, v2.

Computation (reference):
    neigh = x[ind]                                   # [n, k, d] gather
    feat  = [neigh - center, center]                 # [n, k, 2d]
    h     = relu(feat @ W1 + b1) @ W2 + b2           # [n, k, H]
    out   = max over k                               # [n, H]

Key ideas vs v1 (which used per-128-row indirect_dma_start gathers and a DMA
transpose; GPSIMD descriptor generation was 67% busy and the bottleneck):

  - One `dma_gather(transpose=True)` per 512-point block fetches all 8192
    neighbor rows straight into a feature-major slab [128, 8192] -- the gather
    and the transpose are fused into a single SWDGE instruction (994ns fixed
    cost amortized over 8192 rows instead of 64 separate indirect DMAs).
  - dma_gather indices are int16 and rows must be 256B, so the host stages a
    per-block COMPACTED table: unique neighbor x-rows of that block, padded to
    [x_j | zeros] 128 bf16 = 256B.  ~7.9k distinct rows per block << 32767.
  - Edges are laid out K-MAJOR within a block (col = k*NB + pt) so the center
    half of the slab (partitions 64..127) is written by one DVE copy with a
    stride-0 broadcast over k (out [64, K, NB] <- in [64, 1, NB]).
  - mm1 uses the repacked stationary [[W1[:d]], [W1[d:]-W1[:d]]], so
    slab = [neigh | center] needs no subtraction; b1 is the ACT relu bias.
  - k-max is a pairwise tensor_tensor(max) of neighboring k-stripe PSUM tiles
    (fp32, stage 1) followed by a bf16 SBUF max tree (stages 2-4), avoiding a
    full fp32 tensor_reduce pass over PSUM.
  - b2 is added on the host after the max (max(h)+b2 == max(h+b2)); output is
    returned bf16 feature-major and transposed/cast on the host.
"""

import os
import sys

for _p in ("/opt/trn_rl_repo",):
    if _p not in sys.path and os.path.isdir(_p):
        sys.path.insert(0, _p)

import numpy as np
import ml_dtypes

BF16 = ml_dtypes.bfloat16

# problem constants (hardcoded per harness contract)
N, D, K, H = 100000, 64, 16, 128
NCORES = 8
NP = 12500             # points per core
NB = 512               # points per block
EB = NB * K            # edges (gather indices) per block = 8192
TR = 8704              # compacted-table rows per block (>= max distinct + pad)


class Cfg:
    def __init__(self, n=N, np_=NP, nb=NB, tr=TR, gchunk=None,
                 single_packet=False):
        self.n = n
        self.np = np_                   # points handled by this core (unpadded)
        self.nb = nb                    # points per block
        self.eb = nb * K
        self.nblk = -(-np_ // nb)       # ceil
        self.npp = self.nblk * nb       # padded points per core
        self.tr = tr
        # gather call chunking: max indices per dma_gather call (multiple of
        # 128); None = whole block in one call
        self.gchunk = gchunk or self.eb
        self.single_packet = single_packet


def build_program(cfg: Cfg, debug=False):
    import concourse.bacc as bacc
    import concourse.bass as bass
    import concourse.tile as tile
    from concourse import mybir

    f32 = mybir.dt.float32
    bf16 = mybir.dt.bfloat16
    i16 = mybir.dt.int16
    NBK = cfg.nblk
    NBc = cfg.nb
    EBc = cfg.eb
    TRc = cfg.tr

    # two SWDGE queues: consecutive blocks' gathers alternate queues, so a
    # gather's descriptor-ring await never waits on the previous gather's
    # transfer tail (each queue has its own descriptor rings and Q7 core pair)
    nc = bacc.Bacc("TRN2", target_bir_lowering=False, debug=debug,
                   dynamic_dma_scratch_size=32768)

    xtab = nc.dram_tensor("xtab", (NBK, TRc, 2 * D), bf16, kind="ExternalInput")
    idxt = nc.dram_tensor("idxt", (NBK, 128, EBc // 16), i16, kind="ExternalInput")
    xot = nc.dram_tensor("xot", (D, cfg.npp), bf16, kind="ExternalInput")
    w1 = nc.dram_tensor("w1", (2 * D, H), bf16, kind="ExternalInput")
    w2 = nc.dram_tensor("w2", (H, H), bf16, kind="ExternalInput")
    b1 = nc.dram_tensor("b1", (H, 1), f32, kind="ExternalInput")
    out2 = nc.dram_tensor("out2", (H, cfg.npp), bf16, kind="ExternalOutput")

    with tile.TileContext(nc) as tc:
        with (
            tc.tile_pool(name="const", bufs=1) as constp,
            tc.tile_pool(name="idx", bufs=3) as idxp,
            tc.tile_pool(name="xo", bufs=3) as xop,
            tc.tile_pool(name="slab", bufs=3) as slabp,
            tc.tile_pool(name="h1", bufs=3) as h1p,
            tc.tile_pool(name="s2", bufs=2) as s2p,
            tc.tile_pool(name="mx", bufs=2) as mxp,
            tc.tile_pool(name="mx4", bufs=2) as mx4p,
            tc.tile_pool(name="mx2", bufs=2) as mx2p,
            tc.tile_pool(name="outs", bufs=2) as outp,
            tc.tile_pool(name="ps1", bufs=2, space="PSUM") as ps1p,
            tc.tile_pool(name="ps2", bufs=2, space="PSUM") as ps2p,
        ):
            w1s = constp.tile([2 * D, H], bf16)
            nc.sync.dma_start(w1s[:], w1[:, :])
            w2s = constp.tile([H, H], bf16)
            nc.sync.dma_start(w2s[:], w2[:, :])
            b1s = constp.tile([H, 1], f32)
            nc.sync.dma_start(b1s[:], b1[:, :])

            for m in range(NBK):
                idx = idxp.tile([128, EBc // 16], i16)
                nc.sync.dma_start(idx[:], idxt[m])
                xo = xop.tile([D, NBc], bf16)
                nc.sync.dma_start(xo[:], xot[:, m * NBc:(m + 1) * NBc])

                slab = slabp.tile([128, EBc], bf16)
                # single_packet=True wedges the device above ~992 indices
                # (>64 descriptors per SDMA engine in one packet); use the
                # multi-packet path.
                for off in range(0, EBc, cfg.gchunk):
                    n = min(cfg.gchunk, EBc - off)
                    nc.gpsimd.dma_gather(
                        out_ap=slab[:, off:off + n].rearrange(
                            "p (a b) -> p a b", a=1),
                        in_ap=xtab[m],
                        idxs_ap=idx[:, off // 16:(off + n) // 16],
                        num_idxs=n,
                        num_idxs_reg=n,
                        elem_size=2 * D,
                        transpose=True,
                        single_packet=cfg.single_packet,
                    )
                # centers into the slab's upper half, broadcast over k
                nc.vector.tensor_copy(
                    out=slab[D:2 * D, :].rearrange("p (k b) -> p k b", k=K),
                    in_=xo[:].unsqueeze(1).broadcast_to((D, K, NBc)),
                )

                mx = mxp.tile([H, K // 2, NBc], bf16)
                for t in range(K // 2):
                    p1 = ps1p.tile([H, 2, NBc], f32)
                    nc.tensor.matmul(
                        p1[:, 0], lhsT=w1s[:],
                        rhs=slab[:, (2 * t) * NBc:(2 * t + 1) * NBc],
                        start=True, stop=True,
                    )
                    nc.tensor.matmul(
                        p1[:, 1], lhsT=w1s[:],
                        rhs=slab[:, (2 * t + 1) * NBc:(2 * t + 2) * NBc],
                        start=True, stop=True,
                    )
                    h1 = h1p.tile([H, 2, NBc], bf16)
                    nc.scalar.activation(
                        h1[:], p1[:], mybir.ActivationFunctionType.Relu,
                        bias=b1s[:], scale=1.0,
                    )
                    p2 = ps2p.tile([H, 2, NBc], f32)
                    nc.tensor.matmul(p2[:, 0], lhsT=w2s[:], rhs=h1[:, 0],
                                     start=True, stop=True)
                    nc.tensor.matmul(p2[:, 1], lhsT=w2s[:], rhs=h1[:, 1],
                                     start=True, stop=True)
                    # k-pair max; DVE TensorTensor cannot take two PSUM
                    # operands, so split pair-max between an ACT-evac route
                    # (ACT copy to bf16, DVE 4x max) and a DVE strided
                    # tensor_reduce route to balance ACT/DVE load.
                    if t < 3:
                        s2 = s2p.tile([H, 2, NBc], bf16)
                        nc.scalar.activation(
                            s2[:], p2[:], mybir.ActivationFunctionType.Copy,
                        )
                        nc.vector.tensor_tensor(
                            out=mx[:, t], in0=s2[:, 0], in1=s2[:, 1],
                            op=mybir.AluOpType.max,
                        )
                    else:
                        nc.vector.tensor_reduce(
                            out=mx[:, t],
                            in_=p2[:].transpose([0, 2, 1]),
                            axis=mybir.AxisListType.X,
                            op=mybir.AluOpType.max,
                        )

                mx4 = mx4p.tile([H, 4, NBc], bf16)
                nc.vector.tensor_tensor(out=mx4[:], in0=mx[:, 0:4],
                                        in1=mx[:, 4:8], op=mybir.AluOpType.max)
                mx2 = mx2p.tile([H, 2, NBc], bf16)
                nc.vector.tensor_tensor(out=mx2[:], in0=mx4[:, 0:2],
                                        in1=mx4[:, 2:4], op=mybir.AluOpType.max)
                outt = outp.tile([H, NBc], bf16)
                nc.vector.tensor_tensor(out=outt[:], in0=mx2[:, 0],
                                        in1=mx2[:, 1], op=mybir.AluOpType.max)
                nc.sync.dma_start(out2[:, m * NBc:(m + 1) * NBc], outt[:])

    nc.compile()
    return nc


def host_prep(x, W1, b1, W2, b2):
    """Shared (core-independent) input prep."""
    xb = np.ascontiguousarray(x.astype(BF16))
    what = np.vstack([W1[:D], W1[D:] - W1[:D]]).astype(BF16)
    w2b = W2.astype(BF16)
    b1c = np.ascontiguousarray(b1.astype(np.float32).reshape(H, 1))
    return xb, what, w2b, b1c


def core_inputs(cfg: Cfg, xb, what, w2b, b1c, ind32, lo, hi):
    """Build one core's input map for its point range [lo, hi)."""
    NBK, NBc, EBc, TRc = cfg.nblk, cfg.nb, cfg.eb, cfg.tr
    indc = np.zeros((cfg.npp, K), np.int64)
    indc[:hi - lo] = ind32[lo:hi]

    xtab = np.zeros((NBK, TRc, 2 * D), BF16)
    idxt = np.empty((NBK, 128, EBc // 16), np.int16)
    for b in range(NBK):
        blk = indc[b * NBc:(b + 1) * NBc]               # [NB, K]
        uniq, inv = np.unique(blk, return_inverse=True)
        r = len(uniq)
        assert r <= TRc, f"block {b}: {r} distinct rows > table {TRc}"
        xtab[b, :r, 0:D] = xb[uniq]
        # k-major edge order: col j = k*NB + pt
        cols = inv.reshape(NBc, K).T.reshape(EBc)       # [EB] int
        lanes = cols.reshape(EBc // 16, 16).T           # [16, EB/16]
        idxt[b] = np.tile(lanes.astype(np.int16), (8, 1))

    xo = np.zeros((D, cfg.npp), BF16)
    xo[:, :hi - lo] = xb[lo:hi].T
    return {
        "xtab": xtab, "idxt": idxt, "xot": np.ascontiguousarray(xo),
        "w1": what, "w2": w2b, "b1": b1c,
    }


_NC_CACHE = {}


def kernel(x, ind, W1, b1, W2, b2):
    from concourse import bass_utils

    cfg = Cfg()
    key = (cfg.n, cfg.np, cfg.nb, cfg.tr)
    if key not in _NC_CACHE:
        _NC_CACHE[key] = build_program(cfg)
    nc = _NC_CACHE[key]

    x = np.asarray(x, np.float32)
    ind32 = np.asarray(ind).astype(np.int64)
    xb, what, w2b, b1c = host_prep(x, np.asarray(W1, np.float32),
                                   np.asarray(b1, np.float32),
                                   np.asarray(W2, np.float32),
                                   np.asarray(b2, np.float32))
    in_maps = []
    for c in range(NCORES):
        lo = c * NP
        hi = min(lo + NP, N)
        in_maps.append(core_inputs(cfg, xb, what, w2b, b1c, ind32, lo, hi))

    res = bass_utils.run_bass_kernel_spmd(nc, in_maps, core_ids=list(range(NCORES)))
    b2f = np.asarray(b2, np.float32).reshape(1, H)
    out = np.empty((N, H), np.float32)
    for c in range(NCORES):
        lo = c * NP
        hi = min(lo + NP, N)
        out[lo:hi] = res.results[c]["out2"].T[:hi - lo].astype(np.float32) + b2f
    return out


# revision 11
# speedup vs baseline: 1.3477x; 1.3477x over previous
"""EdgeConv (gnn_message_passing) Trainium2 Bass kernel, v2.

Computation (reference):
    neigh = x[ind]                                   # [n, k, d] gather
    feat  = [neigh - center, center]                 # [n, k, 2d]
    h     = relu(feat @ W1 + b1) @ W2 + b2           # [n, k, H]
    out   = max over k                               # [n, H]

Key ideas vs v1 (which used per-128-row indirect_dma_start gathers and a DMA
transpose; GPSIMD descriptor generation was 67% busy and the bottleneck):

  - One `dma_gather(transpose=True)` per 512-point block fetches all 8192
    neighbor rows straight into a feature-major slab [128, 8192] -- the gather
    and the transpose are fused into a single SWDGE instruction (994ns fixed
    cost amortized over 8192 rows instead of 64 separate indirect DMAs).
  - dma_gather indices are int16 and rows must be 256B, so the host stages a
    per-block COMPACTED table: unique neighbor x-rows of that block, padded to
    [x_j | zeros] 128 bf16 = 256B.  ~7.9k distinct rows per block << 32767.
  - Edges are laid out K-MAJOR within a block (col = k*NB + pt) so the center
    half of the slab (partitions 64..127) is written by one DVE copy with a
    stride-0 broadcast over k (out [64, K, NB] <- in [64, 1, NB]).
  - mm1 uses the repacked stationary [[W1[:d]], [W1[d:]-W1[:d]]], so
    slab = [neigh | center] needs no subtraction; b1 is the ACT relu bias.
  - k-max is a pairwise tensor_tensor(max) of neighboring k-stripe PSUM tiles
    (fp32, stage 1) followed by a bf16 SBUF max tree (stages 2-4), avoiding a
    full fp32 tensor_reduce pass over PSUM.
  - b2 is added on the host after the max (max(h)+b2 == max(h+b2)); output is
    returned bf16 feature-major and transposed/cast on the host.
"""

import os
import sys

for _p in ("/opt/trn_rl_repo",):
    if _p not in sys.path and os.path.isdir(_p):
        sys.path.insert(0, _p)

import numpy as np
import ml_dtypes

BF16 = ml_dtypes.bfloat16

# problem constants (hardcoded per harness contract)
N, D, K, H = 100000, 64, 16, 128
NCORES = 8
NP = 12500             # points per core
NB = 512               # points per block
EB = NB * K            # edges (gather indices) per block = 8192
TR = 8704              # compacted-table rows per block (>= max distinct + pad)


class Cfg:
    def __init__(self, n=N, np_=NP, nb=NB, tr=TR, gchunk=None,
                 single_packet=False):
        self.n = n
        self.np = np_                   # points handled by this core (unpadded)
        self.nb = nb                    # points per block
        self.eb = nb * K
        self.nblk = -(-np_ // nb)       # ceil
        self.npp = self.nblk * nb       # padded points per core
        self.tr = tr
        # gather call chunking: max indices per dma_gather call (multiple of
        # 128); None = whole block in one call
        self.gchunk = gchunk or self.eb // 2
        self.single_packet = single_packet


def build_program(cfg: Cfg, debug=False):
    import concourse.bacc as bacc
    import concourse.bass as bass
    import concourse.tile as tile
    from concourse import mybir

    f32 = mybir.dt.float32
    bf16 = mybir.dt.bfloat16
    i16 = mybir.dt.int16
    NBK = cfg.nblk
    NBc = cfg.nb
    EBc = cfg.eb
    TRc = cfg.tr

    # two SWDGE queues: consecutive blocks' gathers alternate queues, so a
    # gather's descriptor-ring await never waits on the previous gather's
    # transfer tail (each queue has its own descriptor rings and Q7 core pair)
    nc = bacc.Bacc("TRN2", target_bir_lowering=False, debug=debug)

    xtab = nc.dram_tensor("xtab", (NBK, TRc, 2 * D), bf16, kind="ExternalInput")
    idxt = nc.dram_tensor("idxt", (NBK, 128, EBc // 16), i16, kind="ExternalInput")
    xot = nc.dram_tensor("xot", (D, cfg.npp), bf16, kind="ExternalInput")
    w1 = nc.dram_tensor("w1", (2 * D, H), bf16, kind="ExternalInput")
    w2 = nc.dram_tensor("w2", (H, H), bf16, kind="ExternalInput")
    b1 = nc.dram_tensor("b1", (H, 1), f32, kind="ExternalInput")
    out2 = nc.dram_tensor("out2", (H, cfg.npp), bf16, kind="ExternalOutput")

    with tile.TileContext(nc) as tc:
        with (
            tc.tile_pool(name="const", bufs=1) as constp,
            tc.tile_pool(name="idx", bufs=3) as idxp,
            tc.tile_pool(name="xo", bufs=3) as xop,
            tc.tile_pool(name="slab", bufs=3) as slabp,
            tc.tile_pool(name="h1", bufs=3) as h1p,
            tc.tile_pool(name="s2", bufs=2) as s2p,
            tc.tile_pool(name="mx", bufs=2) as mxp,
            tc.tile_pool(name="mx4", bufs=2) as mx4p,
            tc.tile_pool(name="mx2", bufs=2) as mx2p,
            tc.tile_pool(name="outs", bufs=2) as outp,
            tc.tile_pool(name="ps1", bufs=2, space="PSUM") as ps1p,
            tc.tile_pool(name="ps2", bufs=2, space="PSUM") as ps2p,
        ):
            w1s = constp.tile([2 * D, H], bf16)
            nc.sync.dma_start(w1s[:], w1[:, :])
            w2s = constp.tile([H, H], bf16)
            nc.sync.dma_start(w2s[:], w2[:, :])
            b1s = constp.tile([H, 1], f32)
            nc.sync.dma_start(b1s[:], b1[:, :])

            for m in range(NBK):
                idx = idxp.tile([128, EBc // 16], i16)
                nc.sync.dma_start(idx[:], idxt[m])
                xo = xop.tile([D, NBc], bf16)
                nc.sync.dma_start(xo[:], xot[:, m * NBc:(m + 1) * NBc])

                slab = slabp.tile([128, EBc], bf16)
                # single_packet=True wedges the device above ~992 indices
                # (>64 descriptors per SDMA engine in one packet); use the
                # multi-packet path.
                for off in range(0, EBc, cfg.gchunk):
                    n = min(cfg.gchunk, EBc - off)
                    nc.gpsimd.dma_gather(
                        out_ap=slab[:, off:off + n].rearrange(
                            "p (a b) -> p a b", a=1),
                        in_ap=xtab[m],
                        idxs_ap=idx[:, off // 16:(off + n) // 16],
                        num_idxs=n,
                        num_idxs_reg=n,
                        elem_size=2 * D,
                        transpose=True,
                        single_packet=cfg.single_packet,
                    )
                # centers into the slab's upper half, broadcast over k
                nc.vector.tensor_copy(
                    out=slab[D:2 * D, :].rearrange("p (k b) -> p k b", k=K),
                    in_=xo[:].unsqueeze(1).broadcast_to((D, K, NBc)),
                )

                mx = mxp.tile([H, K // 2, NBc], bf16)
                for t in range(K // 2):
                    p1 = ps1p.tile([H, 2, NBc], f32)
                    nc.tensor.matmul(
                        p1[:, 0], lhsT=w1s[:],
                        rhs=slab[:, (2 * t) * NBc:(2 * t + 1) * NBc],
                        start=True, stop=True,
                    )
                    nc.tensor.matmul(
                        p1[:, 1], lhsT=w1s[:],
                        rhs=slab[:, (2 * t + 1) * NBc:(2 * t + 2) * NBc],
                        start=True, stop=True,
                    )
                    h1 = h1p.tile([H, 2, NBc], bf16)
                    nc.scalar.activation(
                        h1[:], p1[:], mybir.ActivationFunctionType.Relu,
                        bias=b1s[:], scale=1.0,
                    )
                    p2 = ps2p.tile([H, 2, NBc], f32)
                    nc.tensor.matmul(p2[:, 0], lhsT=w2s[:], rhs=h1[:, 0],
                                     start=True, stop=True)
                    nc.tensor.matmul(p2[:, 1], lhsT=w2s[:], rhs=h1[:, 1],
                                     start=True, stop=True)
                    # k-pair max; DVE TensorTensor cannot take two PSUM
                    # operands, so split pair-max between an ACT-evac route
                    # (ACT copy to bf16, DVE 4x max) and a DVE strided
                    # tensor_reduce route to balance ACT/DVE load.
                    if t < 3:
                        s2 = s2p.tile([H, 2, NBc], bf16)
                        nc.scalar.activation(
                            s2[:], p2[:], mybir.ActivationFunctionType.Copy,
                        )
                        nc.vector.tensor_tensor(
                            out=mx[:, t], in0=s2[:, 0], in1=s2[:, 1],
                            op=mybir.AluOpType.max,
                        )
                    else:
                        nc.vector.tensor_reduce(
                            out=mx[:, t],
                            in_=p2[:].transpose([0, 2, 1]),
                            axis=mybir.AxisListType.X,
                            op=mybir.AluOpType.max,
                        )

                mx4 = mx4p.tile([H, 4, NBc], bf16)
                nc.vector.tensor_tensor(out=mx4[:], in0=mx[:, 0:4],
                                        in1=mx[:, 4:8], op=mybir.AluOpType.max)
                mx2 = mx2p.tile([H, 2, NBc], bf16)
                nc.vector.tensor_tensor(out=mx2[:], in0=mx4[:, 0:2],
                                        in1=mx4[:, 2:4], op=mybir.AluOpType.max)
                outt = outp.tile([H, NBc], bf16)
                nc.vector.tensor_tensor(out=outt[:], in0=mx2[:, 0],
                                        in1=mx2[:, 1], op=mybir.AluOpType.max)
                nc.sync.dma_start(out2[:, m * NBc:(m + 1) * NBc], outt[:])

    nc.compile()
    return nc


def host_prep(x, W1, b1, W2, b2):
    """Shared (core-independent) input prep."""
    xb = np.ascontiguousarray(x.astype(BF16))
    what = np.vstack([W1[:D], W1[D:] - W1[:D]]).astype(BF16)
    w2b = W2.astype(BF16)
    b1c = np.ascontiguousarray(b1.astype(np.float32).reshape(H, 1))
    return xb, what, w2b, b1c


def core_inputs(cfg: Cfg, xb, what, w2b, b1c, ind32, lo, hi):
    """Build one core's input map for its point range [lo, hi)."""
    NBK, NBc, EBc, TRc = cfg.nblk, cfg.nb, cfg.eb, cfg.tr
    indc = np.zeros((cfg.npp, K), np.int64)
    indc[:hi - lo] = ind32[lo:hi]

    xtab = np.zeros((NBK, TRc, 2 * D), BF16)
    idxt = np.empty((NBK, 128, EBc // 16), np.int16)
    for b in range(NBK):
        blk = indc[b * NBc:(b + 1) * NBc]               # [NB, K]
        uniq, inv = np.unique(blk, return_inverse=True)
        r = len(uniq)
        assert r <= TRc, f"block {b}: {r} distinct rows > table {TRc}"
        xtab[b, :r, 0:D] = xb[uniq]
        # k-major edge order: col j = k*NB + pt
        cols = inv.reshape(NBc, K).T.reshape(EBc)       # [EB] int
        lanes = cols.reshape(EBc // 16, 16).T           # [16, EB/16]
        idxt[b] = np.tile(lanes.astype(np.int16), (8, 1))

    xo = np.zeros((D, cfg.npp), BF16)
    xo[:, :hi - lo] = xb[lo:hi].T
    return {
        "xtab": xtab, "idxt": idxt, "xot": np.ascontiguousarray(xo),
        "w1": what, "w2": w2b, "b1": b1c,
    }


_NC_CACHE = {}


def kernel(x, ind, W1, b1, W2, b2):
    from concourse import bass_utils

    cfg = Cfg()
    key = (cfg.n, cfg.np, cfg.nb, cfg.tr)
    if key not in _NC_CACHE:
        _NC_CACHE[key] = build_program(cfg)
    nc = _NC_CACHE[key]

    x = np.asarray(x, np.float32)
    ind32 = np.asarray(ind).astype(np.int64)
    xb, what, w2b, b1c = host_prep(x, np.asarray(W1, np.float32),
                                   np.asarray(b1, np.float32),
                                   np.asarray(W2, np.float32),
                                   np.asarray(b2, np.float32))
    in_maps = []
    for c in range(NCORES):
        lo = c * NP
        hi = min(lo + NP, N)
        in_maps.append(core_inputs(cfg, xb, what, w2b, b1c, ind32, lo, hi))

    res = bass_utils.run_bass_kernel_spmd(nc, in_maps, core_ids=list(range(NCORES)))
    b2f = np.asarray(b2, np.float32).reshape(1, H)
    out = np.empty((N, H), np.float32)
    for c in range(NCORES):
        lo = c * NP
        hi = min(lo + NP, N)
        out[lo:hi] = res.results[c]["out2"].T[:hi - lo].astype(np.float32) + b2f
    return out
